# revision 1
# baseline (speedup 1.0000x reference)
"""CodeShell attention block (B=1, S=2048, 32 Q heads / 8 KV heads GQA,
head_dim=128) as a Trainium2 Bass/Tile kernel, tensor-parallel over heads
across 8 NeuronCores.

Sharding: core c owns Q heads 4c..4c+3 and KV head c (Wqkv column shard
[4096,768], Wproj row shard [512,4096]). Each core computes a partial
yT = Wp_c.T @ attn_c in transposed layout; the all-reduce (sum of the 8
partials) + final bias happen on the host after gather.

Device schedule (per core, bf16 matmul operands, fp32 PSUM accumulation):
  Phase 1   qkvT[768,2048] = Wc.T @ hT + b with Wc resident in SBUF (loaded
            once) and hT streamed in 2MB half-chunks; PSUM copied straight
            to bf16 row tiles.
  RoPE      in-place bf16 on the k tile and 4 q tiles as each finishes its
            last column chunk (overlaps the phase-1 tail); V transposed to
            natural [k,d] layout via the DMA XBAR (no PE/ACT time), into a
            129-column-stride tile whose 129th columns are constant 1.0.
  Attention scoresT[k,q] -> Exp on ACT -> block-causal mask -> PV flipped to
            O[q,d] += P.T @ [V | 1]: the ones column accumulates the softmax
            denominator in PSUM col 128 for free (saves the separate
            ones-matmul + broadcast matmul of the old layout). Normalize is
            a per-partition reciprocal + tensor_scalar multiply on DVE, then
            O goes back to [d,q] via DMA transpose for the output GEMM.
            Scores for block j+1 are emitted before PV of block j so the PE
            never waits on the ACT exp.
  Phase 4   yT = Wp.T @ OT is emitted interleaved into the attention stream:
            output chunk c's matmuls are spread between the attention blocks
            of chunk c+1, keeping the PE fed while ACT chews exps. PSUM is
            copied to bf16 on DVE/Pool (alternating) and streamed out.
"""
import numpy as np
import ml_dtypes

import concourse.bass as bass
import concourse.mybir as mybir
import concourse.tile as tile
DEBUG_DUMPS = False
# tuning knobs (sim-swept)
YT_QUEUE = "sync"        # "pool" | "sync"
DRIP_DELAY = 8
H3_FAST = False          # emit h3 OT transpose on scalar queue instead of sync
PV_BOUNDARY_FIRST = False  # emit pending PV before first block of a window
P4_COPY = "dve"          # "both" (alternate DVE/ACT) | "dve" (all on DVE)
from concourse.bass_utils import run_bass_kernel_spmd
from concourse.vector_clock import ScopedClock, VectorClock

BF16NP = ml_dtypes.bfloat16


class PatchedTileContext(tile.TileContext):
    """TileContext whose kernel-tail drain carries at most one sem wait per
    instruction: the public walrus here rejects a Drain with more than one
    sync wait. Waits are front-loaded onto per-processor NOPs on the SP
    queue (which executes in order), leaving the drain itself waitless."""

    def _drain_and_barrier(self, tick_clock, wait_clock):
        nc = self.nc
        gc = tick_clock.global_clock
        n = len(gc)
        for proc in range(n):
            tick = gc[proc]
            if tick <= 0:
                continue
            vec = [0] * n
            vec[proc] = tick
            nop = nc.sync.nop()
            wait_clock.add_sem_waits(nop.ins, ScopedClock({None: VectorClock(vec)}))
        nc.sync.drain()
        nc.all_engine_barrier()
        assert self.sems is not None
        popped = nc._tile_sem_poison_stack.pop()
        assert popped is self._sem_poison
        nc.clear_and_free_semaphores(list(self.sems.allocated().values()))
        nc.all_engine_barrier()


F32 = mybir.dt.float32
BF16 = mybir.dt.bfloat16
AF = mybir.ActivationFunctionType


def _split_multi_waits(nc):
    """The public walrus in this container encodes at most one sync wait and
    one sync update per instruction (one TPB EVENTS struct). Tile's sem
    assignment freely emits several. Split them: extra waits move onto
    same-engine NOPs inserted immediately before the instruction (the
    engine's sequencer processes waits in program order, so semantics are
    identical); extra updates move onto NOPs immediately after."""
    fn = nc.m.functions[0]
    spawned = set()

    def fresh_nop(engine, on_wait, on_update):
        nbi = nc.engines[engine].nop()
        ninst = nbi.ins
        spawned.add(id(ninst))
        ninst.sync_info = mybir.SyncInfo(on_wait=on_wait, on_update=on_update)
        return ninst

    for bb in fn.blocks:
        newlist = []
        for inst in list(bb.instructions):
            if id(inst) in spawned:
                continue  # already placed next to its parent instruction
            si = inst.sync_info
            waits = list(si.on_wait) if si and si.on_wait else []
            updates = list(si.on_update) if si and si.on_update else []
            pre, post = [], []
            if len(waits) > 1:
                for w in waits[:-1]:
                    pre.append(fresh_nop(inst.engine, [w], []))
                waits = [waits[-1]]
            if len(updates) > 1:
                for u in updates[1:]:
                    post.append(fresh_nop(inst.engine, [], [u]))
                updates = [updates[0]]
            if pre or post:
                inst.sync_info = mybir.SyncInfo(on_wait=waits, on_update=updates)
            newlist.extend(pre)
            newlist.append(inst)
            newlist.extend(post)
        bb.instructions[:] = newlist
    # strip the spawned nops from wherever nc appended them originally
    for bb in fn.blocks:
        seen = set()
        kept = []
        for inst in bb.instructions:
            if id(inst) in spawned:
                if id(inst) in seen:
                    continue
                seen.add(id(inst))
            kept.append(inst)
        bb.instructions[:] = kept


H, S, NH, NKV, D = 4096, 2048, 32, 8, 128
HALF = D // 2
NCORES = 8
QH = NH // NCORES      # 4 q heads per core
MC = 4                 # m-chunks of 512 positions
NT = 6                 # qkvT row tiles per core (4 q heads + k + v)
KK = H // 128          # 32 contraction tiles
QC = 4                 # 512-wide q chunks
NJ = S // 128          # 16 key blocks
VSTRIDE = 129          # vnat column stride (128 d cols + 1 ones col)
ROPE_THETA = 10000.0
SCALE = 0.08838834764831845  # 1/sqrt(head_dim), folded into the Exp activation


def _emit_body(nc, tc, aps):
    ht, wq, bq, wp, cost, sint, mask, ident, identf, yt, dbg = aps

    # ---- persistent (per-rep) pools, allocated before the phase-1 scoped
    # pools so their SBUF ranges never alias the streamed ht buffers ----
    cpool = tc.alloc_tile_pool(name="const", bufs=1)
    bq_sb = cpool.tile([128, NT], F32, tag="bq", name="bq_sb")
    nc.sync.dma_start(bq_sb, bq)
    cos_sb = cpool.tile([128, S], BF16, tag="cost", name="cos_sb")
    nc.sync.dma_start(cos_sb, cost)
    sin_sb = cpool.tile([128, S], BF16, tag="sint", name="sin_sb")
    nc.sync.dma_start(sin_sb, sint)
    mask_sb = cpool.tile([128, 128], BF16, tag="mask", name="mask_sb")
    nc.sync.dma_start(mask_sb, mask)
    id_sb = cpool.tile([128, 128], BF16, tag="ident", name="id_sb")
    nc.sync.dma_start(id_sb, ident)
    idf_sb = cpool.tile([128, 128], F32, tag="identf", name="idf_sb")
    nc.sync.dma_start(idf_sb, identf)

    qkvp = tc.alloc_tile_pool(name="qkvT", bufs=1)
    qkv = [qkvp.tile([128, S], BF16, tag=f"qkv{nt}", name=f"qkv{nt}")
           for nt in range(NT)]

    shp = tc.alloc_tile_pool(name="ropesh", bufs=2)
    m1p = tc.alloc_tile_pool(name="ropem1", bufs=2)

    vpool = tc.alloc_tile_pool(name="vnat", bufs=1)
    vext = vpool.tile([128, NJ * VSTRIDE], BF16, tag="vext", name="vext")

    otp = tc.alloc_tile_pool(name="OT", bufs=1)
    OT = [otp.tile([128, S], BF16, tag=f"OT{hh}", name=f"OT{hh}")
          for hh in range(QH)]

    wqp = tc.alloc_tile_pool(name="wqp", bufs=1)
    wq_sb = [wqp.tile([128, H], BF16, tag=f"wq{nt}", name=f"wq{nt}")
             for nt in range(NT)]
    wpp = tc.alloc_tile_pool(name="wpp", bufs=1)
    wp_sb = [wpp.tile([128, 512], BF16, tag=f"wp{nt}", name=f"wp{nt}")
             for nt in range(32)]

    ptp = tc.alloc_tile_pool(name="ptp", bufs=6)
    osp = tc.alloc_tile_pool(name="osb", bufs=4)
    rp = tc.alloc_tile_pool(name="rp", bufs=4)
    ysp = tc.alloc_tile_pool(name="ysb", bufs=3)

    # ---- RoPE: in-place bf16 on one 512-column chunk of a row tile. Chunk 0
    # of every tile runs at the phase-1 tail; chunk c+1 is dripped through
    # attention window c so the rope ops never clog the DVE/SP queues ahead
    # of the masks/normalizes the PE is waiting on. sh holds the swapped
    # halves (sh[0:64]=x[64:128], sh[64:128]=x[0:64]) via SBUF->SBUF DMA
    # (compute operands must share partition ranges on this walrus).
    def rope_chunk(ntt, c):
        x = qkv[ntt]
        sl = slice(c * 512, (c + 1) * 512)
        sh = shp.tile([128, 512], BF16, tag="sh", name="rope_sh")
        m1 = m1p.tile([128, 512], BF16, tag="m1", name="rope_m1")
        nc.sync.dma_start(sh[0:64], x[64:128, sl])
        nc.sync.dma_start(sh[64:128], x[0:64, sl])
        nc.vector.tensor_mul(m1[0:64], x[0:64, sl], cos_sb[0:64, sl])
        nc.vector.tensor_mul(sh[0:64], sh[0:64], sin_sb[0:64, sl])
        nc.vector.tensor_mul(m1[64:128], x[64:128, sl], cos_sb[64:128, sl])
        nc.vector.tensor_mul(sh[64:128], sh[64:128], sin_sb[64:128, sl])
        nc.vector.tensor_sub(x[0:64, sl], m1[0:64], sh[0:64])
        nc.vector.tensor_add(x[64:128, sl], m1[64:128], sh[64:128])

    # k and v row-tiles first so RoPE(k) / V-transpose / attention feed in
    # while the last q tiles are still on the PE.
    NT_ORDER = [QH, QH + 1, 0, 1, 2, 3]

    # ---- Phase 1: qkvT = Wc.T @ hT + b, W resident, hT streamed ----
    with tc.tile_pool(name="htp", bufs=3) as htp, \
         tc.tile_pool(name="qkps", bufs=2, space="PSUM") as qkp, \
         tc.tile_pool(name="vtps", bufs=2, space="PSUM") as vtp:
        ht_half = {}

        def load_half(mc, half):
            t = htp.tile([128, 16 * 512], BF16, tag="ht", name=f"ht{mc}_{half}")
            nc.scalar.dma_start(t, ht[mc, half])
            ht_half[(mc, half)] = t

        # priority: first ht half + k/v weight strips, then the rest
        load_half(0, 0)
        nc.scalar.dma_start(wq_sb[NT_ORDER[0]], wq[NT_ORDER[0]])
        load_half(0, 1)
        nc.scalar.dma_start(wq_sb[NT_ORDER[1]], wq[NT_ORDER[1]])
        for nt in NT_ORDER[2:]:
            nc.scalar.dma_start(wq_sb[nt], wq[nt])

        for mc in range(MC):
            if mc + 1 < MC:
                load_half(mc + 1, 0)
                load_half(mc + 1, 1)
            # wp prefetch rides the idle SP queue during phase 1
            for nt8 in range(8):
                w = mc * 8 + nt8
                nc.sync.dma_start(wp_sb[w], wp[w])
            for nt in NT_ORDER:
                ps = qkp.tile([128, 512], F32, tag="ps", name="qk_ps")
                for k in range(KK):
                    t = ht_half[(mc, k // 16)]
                    rhs = t[:, (k % 16) * 512:(k % 16) * 512 + 512]
                    nc.tensor.matmul(ps, wq_sb[nt][:, k * 128:(k + 1) * 128],
                                     rhs, start=(k == 0), stop=(k == KK - 1))
                nc.scalar.activation(qkv[nt][:, mc * 512:(mc + 1) * 512], ps,
                                     AF.Identity, bias=bq_sb[:, nt:nt + 1])
                if mc == MC - 1:
                    # row tile complete: rope / v-transpose it while the
                    # remaining strips are still on the PE
                    if nt == QH:
                        rope_chunk(QH, 0)
                    elif nt == QH + 1:
                        for j in range(NJ):
                            nc.vector.memset(
                                vext[:, j * VSTRIDE + 128:j * VSTRIDE + VSTRIDE], 1.0)
                        # the XBAR DMA transpose corrupts on dst offsets that
                        # are not 128-col aligned (vext blocks sit at stride
                        # 129) — transpose on the PE instead, copies on ACT
                        for j in range(NJ):
                            tp = vtp.tile([128, 128], BF16, tag="vt",
                                          name="vt_ps")
                            nc.tensor.transpose(
                                tp, qkv[QH + 1][:, j * 128:(j + 1) * 128],
                                id_sb)
                            nc.scalar.copy(
                                vext[:, j * VSTRIDE:j * VSTRIDE + 128], tp)
                    else:
                        rope_chunk(nt, 0)

    # ---- Attention + interleaved output projection ----
    kT = qkv[QH]
    with tc.tile_pool(name="stps", bufs=2, space="PSUM") as stp, \
         tc.tile_pool(name="oqps", bufs=4, space="PSUM") as oqp, \
         tc.tile_pool(name="yps", bufs=2, space="PSUM") as ypp:
        ystage = [None]
        deferred_yt = []
        pending_copy = [None]

        def emit_p4_group(qc, nt):
            yp = ypp.tile([128, 512], F32, tag="yp", name="y_ps")
            for kb in range(QH):
                nc.tensor.matmul(yp, wp_sb[nt][:, kb * 128:(kb + 1) * 128],
                                 OT[kb][:, qc * 512:(qc + 1) * 512],
                                 start=(kb == 0), stop=(kb == QH - 1))
            # PSUM can only be read by ACT/DVE; the copy is emitted one group
            # late (its matmuls long done) so it never stalls either queue's
            # head in front of the attention exp/mask/normalize ops.
            if pending_copy[0] is not None:
                pending_copy[0]()
            if nt % 4 == 0:
                ystage[0] = ysp.tile([128, 2048], BF16, tag="ys", name="y_sb")

            def do_copy(qc=qc, nt=nt, yp=yp, st_tile=ystage[0]):
                if nt % 2 == 0 or P4_COPY == "dve":
                    nc.vector.tensor_copy(
                        st_tile[:, (nt % 4) * 512:(nt % 4 + 1) * 512], yp)
                else:
                    nc.scalar.copy(
                        st_tile[:, (nt % 4) * 512:(nt % 4 + 1) * 512], yp)
                if nt % 4 == 3:
                    # stores ride the otherwise-idle Pool SWDGE queue (legal:
                    # only PSUM access is banned on gpsimd, this is SBUF->
                    # DRAM) so they can't head-of-line block the SP queue
                    if YT_QUEUE == "pool":
                        nc.gpsimd.dma_start(yt[qc, nt // 4], st_tile)
                    else:
                        deferred_yt.append([3, lambda q=qc, g=nt // 4,
                                            t=st_tile:
                                            nc.sync.dma_start(yt[q, g], t)])

            pending_copy[0] = do_copy

        oqd_cur = [None] * 4
        pending_tr = []

        def emit_pv(qc, h, j, off, pt):
            # flush last block's O transposes first: their ob (DVE) is ready
            # by now, the PE transpose runs in-queue, and the DVE copy puts
            # OT in SBUF ~1.5us after the PV stop — no DMA-transpose queueing
            for (h2, qc2, qg2, ob2, src2) in pending_tr:
                nc.tensor.transpose(src2[:, 0:128], ob2, idf_sb)
                nc.vector.tensor_copy(
                    OT[h2][:, qc2 * 512 + qg2 * 128:
                           qc2 * 512 + (qg2 + 1) * 128],
                    src2[:, 0:128])
            pending_tr.clear()
            if j == 0:
                # one PSUM bank per accumulation group: two groups sharing a
                # bank corrupts the first group's region on real HW
                for qg in range(4):
                    oqd_cur[qg] = oqp.tile([128, VSTRIDE], F32, tag="oq",
                                           name="oq_ps")
            for qg in range(off // 128, 4):
                src = oqd_cur[qg]
                nc.tensor.matmul(
                    src, pt[:, qg * 128:(qg + 1) * 128],
                    vext[:, j * VSTRIDE:(j + 1) * VSTRIDE],
                    start=(j == 0), stop=(j == 4 * qc + qg))
                if j == 4 * qc + qg:
                    rr = rp.tile([128, 1], F32, tag="r", name="rr")
                    nc.vector.reciprocal(rr, src[:, 128:129])
                    ob = osp.tile([128, 128], F32, tag="ob", name="ob")
                    nc.vector.tensor_scalar_mul(ob, src[:, 0:128], rr)
                    # the closed accumulator bank doubles as the transpose
                    # scratch (cols 0:128, f32) — no extra PSUM bank needed
                    pending_tr.append((h, qc, qg, ob, src))

        # one flat software-pipelined stream over (qc, h, j): the PV of a
        # block is emitted behind the scores+exp of the NEXT block (even
        # across head/chunk boundaries) so the PE never sits behind an exp;
        # output-projection chunk qc-1 is drip-fed between blocks of qc.
        prev = None
        for qc in range(QC):
            nj = 4 * qc + 4
            nblocks = QH * nj
            ngroups = 32 if qc >= 1 else 0  # chunk qc-1 of the out-proj
            gi = 0
            # the last OT tile of chunk qc-1 (head 3) only lands a couple us
            # into this window — hold the drip back a few blocks
            delay = DRIP_DELAY
            per_block = (ngroups / (nblocks - delay)) if qc >= 1 else 0.0
            acc = -delay * per_block
            # rope chunk qc+1 of k and the q heads, dripped between blocks
            rope_tasks = [QH, 0, 1, 2, 3] if qc + 1 < QC else []
            rope_every = max(1, nblocks // (len(rope_tasks) + 1)) \
                if rope_tasks else 0
            bi = 0
            for h in range(QH):
                qT = qkv[h]
                for j in range(nj):
                    # diagonal blocks: columns below the block diagonal are
                    # non-causal for every k-row in the block — skip them;
                    # only the [off,off+128) strip needs the triangular mask.
                    diag = j >= 4 * qc
                    off = (j - 4 * qc) * 128 if diag else 0
                    if PV_BOUNDARY_FIRST and h == 0 and j == 0 \
                            and prev is not None:
                        emit_pv(*prev)
                        prev = None
                    st = stp.tile([128, 512], F32, tag="st", name="st_ps")
                    nc.tensor.matmul(st[:, off:], kT[:, j * 128:(j + 1) * 128],
                                     qT[:, qc * 512 + off:(qc + 1) * 512],
                                     start=True, stop=True)
                    pt = ptp.tile([128, 512], BF16, tag="pt", name="pt")
                    nc.scalar.activation(pt[:, off:], st[:, off:], AF.Exp,
                                         scale=SCALE)
                    if diag:
                        nc.vector.tensor_mul(pt[:, off:off + 128],
                                             pt[:, off:off + 128], mask_sb)
                    if prev is not None:
                        emit_pv(*prev)
                    for dd in deferred_yt:
                        dd[0] -= 1
                    while deferred_yt and deferred_yt[0][0] <= 0:
                        deferred_yt.pop(0)[1]()
                    prev = (qc, h, j, off, pt)
                    acc += per_block
                    while gi < ngroups and acc >= gi + 1:
                        emit_p4_group(qc - 1, gi)
                        gi += 1
                    bi += 1
                    if rope_tasks and bi % rope_every == 0:
                        rope_chunk(rope_tasks.pop(0), qc + 1)
            if qc == QC - 1 and prev is not None:
                # flush the last PV (and its OT transposes) ahead of the
                # leftover out-proj groups so the final chunk's matmuls
                # aren't queued behind ops that can't complete yet
                emit_pv(*prev)
                prev = None
            while gi < ngroups:
                emit_p4_group(qc - 1, gi)
                gi += 1
            while rope_tasks:
                rope_chunk(rope_tasks.pop(0), qc + 1)
        if prev is not None:
            emit_pv(*prev)
        for (h2, qc2, qg2, ob2, src2) in pending_tr:
            nc.tensor.transpose(src2[:, 0:128], ob2, idf_sb)
            nc.vector.tensor_copy(
                OT[h2][:, qc2 * 512 + qg2 * 128:qc2 * 512 + (qg2 + 1) * 128],
                src2[:, 0:128])
        pending_tr.clear()

        # last output chunk has nothing to hide behind
        for nt in range(32):
            emit_p4_group(QC - 1, nt)
            while deferred_yt:
                deferred_yt.pop(0)[1]()
        if pending_copy[0] is not None:
            pending_copy[0]()
            pending_copy[0] = None
        while deferred_yt:
            deferred_yt.pop(0)[1]()

    if dbg is not None:
        qdump, vdump, odump = dbg
        for nt in range(NT):
            nc.sync.dma_start(qdump[nt], qkv[nt])
        nc.sync.dma_start(vdump, vext)
        for hh in range(QH):
            nc.sync.dma_start(odump[hh], OT[hh])

    for p in (ysp, rp, osp, ptp, wpp, wqp, otp, vpool, m1p, shp, qkvp, cpool):
        p.release()


def build_program(reps=1):
    nc = bass.Bass("TRN2", target_bir_lowering=False, debug=False)
    ht = nc.dram_tensor("ht", [MC, 2, 128, 16, 512], BF16, kind="ExternalInput").ap()
    wq = nc.dram_tensor("wq", [NT, 128, H], BF16, kind="ExternalInput").ap()
    bq = nc.dram_tensor("bq", [128, NT], F32, kind="ExternalInput").ap()
    wp = nc.dram_tensor("wp", [32, 128, 512], BF16, kind="ExternalInput").ap()
    cost = nc.dram_tensor("cost", [128, S], BF16, kind="ExternalInput").ap()
    sint = nc.dram_tensor("sint", [128, S], BF16, kind="ExternalInput").ap()
    mask = nc.dram_tensor("mask", [128, 128], BF16, kind="ExternalInput").ap()
    ident = nc.dram_tensor("ident", [128, 128], BF16, kind="ExternalInput").ap()
    identf = nc.dram_tensor("identf", [128, 128], F32, kind="ExternalInput").ap()
    yt = nc.dram_tensor("yt", [QC, 8, 128, 2048], BF16, kind="ExternalOutput").ap()
    dbg = None
    if DEBUG_DUMPS:
        qdump = nc.dram_tensor("qdump", [NT, 128, S], BF16, kind="ExternalOutput").ap()
        vdump = nc.dram_tensor("vdump", [128, NJ * VSTRIDE], BF16, kind="ExternalOutput").ap()
        odump = nc.dram_tensor("odump", [QH, 128, S], BF16, kind="ExternalOutput").ap()
        dbg = (qdump, vdump, odump)
    aps = (ht, wq, bq, wp, cost, sint, mask, ident, identf, yt, dbg)

    ctx_lp = nc.allow_low_precision(reason="bf16 matmul operands; fp32 PSUM accumulation")
    ctx_lp.__enter__()
    with PatchedTileContext(nc) as tc:
        for _rep in range(reps):
            _emit_body(nc, tc, aps)
    ctx_lp.__exit__(None, None, None)
    _split_multi_waits(nc)
    return nc


def host_prep(positions, hidden_states, Wqkv, bqkv, Wproj):
    pos = np.asarray(positions).reshape(S)
    h = np.asarray(hidden_states, dtype=np.float32).reshape(S, H)
    Wqkv = np.asarray(Wqkv, dtype=np.float32)
    bqkv = np.asarray(bqkv, dtype=np.float32)
    Wproj = np.asarray(Wproj, dtype=np.float32)

    # ht[mc, half, p, kk, c] = h[mc*512 + c, (half*16 + kk)*128 + p]
    ht_t = np.ascontiguousarray(
        h.reshape(MC, 512, 2, 16, 128).transpose(0, 2, 4, 3, 1)).astype(BF16NP)

    inv_freq = (np.float32(1.0) / (np.float32(ROPE_THETA) **
                (np.arange(HALF, dtype=np.float32) / np.float32(HALF)))).astype(np.float32)
    ang = pos.astype(np.float32)[:, None] * inv_freq[None, :]
    cos = np.cos(ang).astype(np.float32).T     # [64, S]
    sin = np.sin(ang).astype(np.float32).T
    cost = np.ascontiguousarray(np.concatenate([cos, cos], axis=0)).astype(BF16NP)
    sint = np.ascontiguousarray(np.concatenate([sin, sin], axis=0)).astype(BF16NP)

    dk = np.arange(128)[:, None]
    dq = np.arange(128)[None, :]
    mask = np.ascontiguousarray((dq >= dk).astype(np.float32)).astype(BF16NP)
    identf = np.eye(128, dtype=np.float32)
    ident = identf.astype(BF16NP)

    shared = {"ht": ht_t, "cost": cost, "sint": sint, "mask": mask,
              "ident": ident, "identf": identf}
    per_core = []
    for c in range(NCORES):
        Wc = np.concatenate([
            Wqkv[:, 512 * c: 512 * (c + 1)],
            Wqkv[:, H + 128 * c: H + 128 * (c + 1)],
            Wqkv[:, H + 1024 + 128 * c: H + 1024 + 128 * (c + 1)],
        ], axis=1)
        bc = np.concatenate([
            bqkv[512 * c: 512 * (c + 1)],
            bqkv[H + 128 * c: H + 128 * (c + 1)],
            bqkv[H + 1024 + 128 * c: H + 1024 + 128 * (c + 1)],
        ])
        wq_t = np.ascontiguousarray(
            Wc.reshape(KK, 128, NT, 128).transpose(2, 1, 0, 3).reshape(NT, 128, H)).astype(BF16NP)
        bq_t = np.ascontiguousarray(bc.reshape(NT, 128).T)
        Wp_c = Wproj[512 * c: 512 * (c + 1), :]
        wp_t = np.ascontiguousarray(
            Wp_c.reshape(QH, 128, 32, 128).transpose(2, 1, 0, 3).reshape(32, 128, 512)).astype(BF16NP)
        per_core.append({"wq": wq_t, "bq": bq_t, "wp": wp_t})
    return shared, per_core


_NC = None


def _get_nc():
    global _NC
    if _NC is None:
        _NC = build_program()
    return _NC


def kernel(positions, hidden_states, Wqkv, bqkv, Wproj, bproj):
    shared, per_core = host_prep(positions, hidden_states, Wqkv, bqkv, Wproj)
    nc = _get_nc()
    in_maps = [dict(shared, **per_core[c]) for c in range(NCORES)]
    res = run_bass_kernel_spmd(nc, in_maps, core_ids=list(range(NCORES)))
    acc = np.zeros((S, H), np.float32)
    for c in range(NCORES):
        # yt[qc, g, p, i]: hidden row (4g + i//512)*128 + p, col qc*512 + i%512
        v = res.results[c]["yt"].astype(np.float32).reshape(QC, 8, 128, 4, 512)
        # -> [qc, 512(c), 8(g), 4(nt), 128(p)] = [S, H] after reshape
        acc += v.transpose(0, 4, 1, 3, 2).reshape(S, H)
    y = acc + np.asarray(bproj, dtype=np.float32)[None, :]
    return y.reshape(1, S, H).astype(np.float32)

